# revision 1
# baseline (speedup 1.0000x reference)
"""Causal self-attention (T=2048, D=2048, H=16) on 8 Trainium2 NeuronCores.

Head-sharded tensor parallel, row-parallel output projection: 2 heads per
core. Each core computes its heads' q/k/v projections and causal
attention; the output projection is computed as a PARTIAL sum over this
core's 256 attention features (y_partial = W_proj[:, rows_r] @ o_r), and
a chunked ReduceScatter sums the partials and hands core r its 256
output-feature rows. This keeps the projection operands SBUF-resident
(no AllGather + 16MB DRAM reload) and moves half the bytes.

All storage/matmul dtypes are bf16 (PE streams 1 col/cycle for bf16 and
fp32r alike, but bf16 halves DMA + collective traffic and enables FWL
for weight loads); PSUM accumulation is fp32.

Layouts (feature/d-major so the PE contracts along partitions):
  - xT      [D, T]     : x transposed (host-side), bf16
  - wqkvT   [D, 768]   : this core's W_attn rows (q0 q1 k0 k1 v0 v1), T
  - wpT     [256, D]   : W_proj[:, rows_r].T (input-dim slice), bf16
  - qT/kT   [128, T] per head (feature on partition)
  - v       [tok, 256] (token on partition) so P.T@V needs no transpose
  - S_T     [tk, tq] so softmax sums run via an all-ones matmul on the PE
"""

import numpy as np
import ml_dtypes

import concourse.bacc as bacc
import concourse.bass_utils as bass_utils
import concourse.mybir as mybir
import concourse.tile as tile

T = 2048
D = 2048
H = 16
C = 128
N_CORES = 8
HPC = H // N_CORES          # heads per core = 2
FPC = HPC * C               # features per core = 256
TQB = 512                   # tq block (PSUM free-dim limit for fp32)
NTQ = T // TQB              # 4
NKT = T // 128              # 16 tk tiles
ND = D // 128               # 16 contraction tiles
SCALE = 1.0 / np.sqrt(np.float32(C))

BF = mybir.dt.bfloat16
F32 = mybir.dt.float32
NPBF = ml_dtypes.bfloat16

_NC_CACHE = {}


def build_nc(sim_single_core=False, reps=1, phases=3, rs_mode="chunk"):
    key = ("sim" if sim_single_core else "nc") + f"_{reps}_{phases}_{rs_mode}"
    if key in _NC_CACHE:
        return _NC_CACHE[key]
    ndev = 1 if sim_single_core else N_CORES
    nc = bacc.Bacc("TRN2", target_bir_lowering=False, debug=False, num_devices=ndev)

    xT = nc.dram_tensor("xT", [D, T], BF, kind="ExternalInput").ap()
    wqkvT = nc.dram_tensor("wqkvT", [D, 3 * FPC], BF, kind="ExternalInput").ap()
    wpT = nc.dram_tensor("wpT", [FPC, D], BF, kind="ExternalInput").ap()
    # mask band: maskB[p, j] = 1.0 if p <= j - 384 else 0.0  (j in [0, 896))
    maskB = nc.dram_tensor("maskB", [128, 896], BF, kind="ExternalInput").ap()
    if phases == 3:
        yT = nc.dram_tensor("yT", [FPC, T], BF, kind="ExternalOutput").ap()
    elif phases == 2:
        yT = nc.dram_tensor("yT", [FPC, T], BF, kind="ExternalOutput").ap()
    else:
        yT = nc.dram_tensor("yT", [128, 6 * T], BF, kind="ExternalOutput").ap()

    if rs_mode == "single":
        ypart_full = nc.dram_tensor("ypart", [D, NTQ * TQB], BF,
                                    kind="Internal").ap()
        yrs_full = nc.dram_tensor("yrs", [FPC, NTQ * TQB], BF,
                                  kind="Internal").ap()
        ypart = [ypart_full[:, j * TQB:(j + 1) * TQB] for j in range(NTQ)]
        yrs = [yrs_full[:, j * TQB:(j + 1) * TQB] for j in range(NTQ)]
        rs_full = (ypart_full, yrs_full)
    else:
        ypart = [nc.dram_tensor(f"ypart{j}", [D, TQB], BF, kind="Internal").ap()
                 for j in range(NTQ)]
        yrs = [nc.dram_tensor(f"yrs{j}", [FPC, TQB], BF, kind="Internal").ap()
               for j in range(NTQ)]
        rs_full = None

    with tile.TileContext(nc) as tc:
        with tc.tile_pool(name="persist", bufs=1) as pp, \
             tc.tile_pool(name="ptiles", bufs=8) as ppt, \
             tc.tile_pool(name="small", bufs=2) as smp, \
             tc.tile_pool(name="psA", bufs=3, space="PSUM") as psA, \
             tc.tile_pool(name="psB", bufs=1, space="PSUM") as psB, \
             tc.tile_pool(name="psC", bufs=1, space="PSUM") as psC:

            mask_sb = pp.tile([128, 896], BF, tag="mask")
            ones_b = pp.tile([128, 128], BF, tag="onesb")
            nc.vector.memset(ones_b[:], 1.0)
            nc.sync.dma_start(mask_sb[:], maskB[:])

            for _rep in range(reps):
                emit_body(nc, tc, pp, ppt, smp, psA, psB, psC,
                          xT, wqkvT, wpT, yT, ypart, yrs,
                          mask_sb, ones_b, sim_single_core, phases,
                          rs_mode, rs_full)

    nc.compile()
    _NC_CACHE[key] = nc
    return nc


def emit_body(nc, tc, pp, ppt, smp, psA, psB, psC,
              xT, wqkvT, wpT, yT, ypart, yrs, mask_sb, ones_b,
              sim_single_core, phases=3, rs_mode="chunk", rs_full=None):
    # ---- phase 1: QKV projections ----
    # qkT layout: feature-block fb in {q_h0, q_h1, k_h0, k_h1} at cols
    # [fb*T, (fb+1)*T); v_sb: tok-tile tt at cols [tt*FPC, ...).
    qkT = pp.tile([128, 4 * T], BF, tag="qkT")               # 16KB/part
    v_sb = pp.tile([128, NKT * FPC], BF, tag="v")            # 8KB/part
    wp_sb = pp.tile([128, HPC * D], BF, tag="wp")            # 8KB/part
    ph1_cm = tc.tile_pool(name="ph1", bufs=1)
    sp_cm = tc.tile_pool(name="stream", bufs=2)
    ph1 = ph1_cm.__enter__()
    sp = sp_cm.__enter__()
    w_sb = ph1.tile([128, ND * 3 * FPC], BF, tag="wbig")     # 24KB/part
    xcols = []
    for tb in range(NTQ):
        xcols.append(sp.tile([128, ND * TQB], BF, tag="xcol", name=f"xcol{tb}"))
    # interleave DMA emission so the first-needed tiles land first
    for t in range(ND):
        nc.sync.dma_start(
            w_sb[:, t * 3 * FPC:(t + 1) * 3 * FPC],
            wqkvT[t * 128:(t + 1) * 128, :])
        nc.sync.dma_start(
            xcols[0][:, t * TQB:(t + 1) * TQB],
            xT[t * 128:(t + 1) * 128, 0:TQB])
    for h in range(HPC):
        nc.sync.dma_start(
            wp_sb[:, h * D:(h + 1) * D], wpT[h * 128:(h + 1) * 128, :])
    for tb in range(NTQ):
        xcol = xcols[tb]
        if tb > 0:
            for t in range(ND):
                nc.sync.dma_start(
                    xcol[:, t * TQB:(t + 1) * TQB],
                    xT[t * 128:(t + 1) * 128, tb * TQB:(tb + 1) * TQB])
        for fb in range(4):
            ps = psA.tile([128, TQB], F32, tag="a")
            for t in range(ND):
                nc.tensor.matmul(
                    ps[:],
                    w_sb[:, t * 3 * FPC + fb * 128: t * 3 * FPC + fb * 128 + 128],
                    xcol[:, t * TQB:(t + 1) * TQB],
                    start=(t == 0), stop=(t == ND - 1))
            nc.vector.tensor_copy(
                qkT[:, fb * T + tb * TQB: fb * T + (tb + 1) * TQB], ps[:])
        for tt in range(4):
            tok = tb * 4 + tt
            ps = psB.tile([128, FPC], F32, tag="b")
            for t in range(ND):
                nc.tensor.matmul(
                    ps[:],
                    xcol[:, t * TQB + tt * 128: t * TQB + (tt + 1) * 128],
                    w_sb[:, t * 3 * FPC + 2 * FPC:(t + 1) * 3 * FPC],
                    start=(t == 0), stop=(t == ND - 1))
            nc.vector.tensor_copy(v_sb[:, tok * FPC:(tok + 1) * FPC], ps[:])

    sp_cm.__exit__(None, None, None)
    ph1_cm.__exit__(None, None, None)

    if phases == 1:
        nc.sync.dma_start(yT[:, 0:4 * T], qkT[:])
        nc.sync.dma_start(yT[:, 4 * T:4 * T + NKT * FPC], v_sb[:])
        return

    # ---- phase 2+3: causal attention, software-pipelined; projection
    # partial + ReduceScatter chunk fire right after each tq block ----
    # tk tiles are PAIRED into [128, 2*TQB] PSUM tiles (2 banks) so one
    # ACT instruction exponentiates two tiles — ACT per-instruction
    # overhead is the attention-phase bottleneck, not PE time.
    # The PE executes its queue in order, so the S pair (pk+LOOKAHEAD)
    # is emitted BEFORE sum/PV of pair pk: while exp(pk) runs on ACT,
    # the PE computes future S tiles instead of stalling.
    LOOKAHEAD = 2
    for j in range(NTQ):
        n_tk = 4 * (j + 1)                  # causal: tk tiles 0..4j+3
        n_pk = n_tk // 2
        sum_ps = {}
        o_ps = {}
        for h in range(HPC):
            sum_ps[h] = psB.tile([128, TQB], F32, tag="b", name=f"sum{j}{h}")
            o_ps[h] = psC.tile([128, TQB], F32, tag="c", name=f"ops{j}{h}")
        p_tiles = {}

        def emit_s_exp_pair(h, pk, j=j):
            qh = qkT[:, h * T:(h + 1) * T]
            kh = qkT[:, (2 + h) * T:(3 + h) * T]
            s_ps = psA.tile([128, 2 * TQB], F32, tag="a", name=f"s{j}{h}{pk}")
            for half in range(2):
                tk = 2 * pk + half
                nc.tensor.matmul(
                    s_ps[:, half * TQB:(half + 1) * TQB],
                    kh[:, tk * 128:(tk + 1) * 128],
                    qh[:, j * TQB:(j + 1) * TQB],
                    start=True, stop=True)
            p_sb = ppt.tile([128, 2 * TQB], BF, tag="p", name=f"p{j}{h}{pk}")
            nc.scalar.activation(
                p_sb[:], s_ps[:], mybir.ActivationFunctionType.Exp,
                scale=float(SCALE))
            for half in range(2):
                tk = 2 * pk + half
                delta = tk * 128 - j * TQB
                if delta >= 0:              # diagonal tile: causal mask
                    nc.vector.tensor_mul(
                        p_sb[:, half * TQB:(half + 1) * TQB],
                        p_sb[:, half * TQB:(half + 1) * TQB],
                        mask_sb[:, 384 - delta: 896 - delta])
            return p_sb

        for pk in range(min(LOOKAHEAD, n_pk)):
            for h in range(HPC):
                p_tiles[h, pk] = emit_s_exp_pair(h, pk)
        for pk in range(n_pk):
            if pk + LOOKAHEAD < n_pk:
                for h in range(HPC):
                    p_tiles[h, pk + LOOKAHEAD] = emit_s_exp_pair(
                        h, pk + LOOKAHEAD)
            for half in range(2):
                tk = 2 * pk + half
                for h in range(HPC):
                    p_sb = p_tiles[h, pk]
                    p_half = p_sb[:, half * TQB:(half + 1) * TQB]
                    nc.tensor.matmul(
                        sum_ps[h][:], ones_b[:], p_half,
                        start=(tk == 0), stop=(tk == n_tk - 1))
                    nc.tensor.matmul(
                        o_ps[h][:],
                        v_sb[:, tk * FPC + h * 128: tk * FPC + (h + 1) * 128],
                        p_half,
                        start=(tk == 0), stop=(tk == n_tk - 1))
            for h in range(HPC):
                p_tiles.pop((h, pk))
        o_sb = {}
        for h in range(HPC):
            inv_sb = smp.tile([128, TQB], F32, tag="inv", name=f"inv{j}{h}")
            nc.vector.reciprocal(inv_sb[:], sum_ps[h][:])
            o_sb[h] = smp.tile([128, TQB], BF, tag="osb", name=f"osb{j}{h}")
            nc.vector.tensor_mul(o_sb[h][:], o_ps[h][:], inv_sb[:])
            if phases == 2:
                nc.sync.dma_start(
                    yT[h * 128:(h + 1) * 128, j * TQB:(j + 1) * TQB], o_sb[h][:])

        if phases == 2:
            continue

        # ---- projection partial for block j (operands SBUF-resident);
        # two o-tiles share one [128, 2*TQB] PSUM tile, copies on the
        # otherwise-idle GPSIMD engine ----
        for op_ in range(ND // 2):
            ps = psA.tile([128, 2 * TQB], F32, tag="a", name=f"yp{j}{op_}")
            for half in range(2):
                ot = 2 * op_ + half
                for h in range(HPC):
                    nc.tensor.matmul(
                        ps[:, half * TQB:(half + 1) * TQB],
                        wp_sb[:, h * D + ot * 128: h * D + (ot + 1) * 128],
                        o_sb[h][:],
                        start=(h == 0), stop=(h == HPC - 1))
            y_sb = smp.tile([128, 2 * TQB], BF, tag="ysb", name=f"ysb{j}{op_}")
            if op_ % 2 == 0:
                nc.vector.tensor_copy(y_sb[:], ps[:])
            else:
                nc.scalar.copy(y_sb[:], ps[:])
            for half in range(2):
                ot = 2 * op_ + half
                nc.sync.dma_start(
                    ypart[j][ot * 128:(ot + 1) * 128, :],
                    y_sb[:, half * TQB:(half + 1) * TQB])

        # ---- ReduceScatter chunk j: sum partials, keep our 256 rows ----
        if rs_mode == "single":
            continue
        if sim_single_core or rs_mode == "none":
            nc.sync.dma_start(yrs[j][:], ypart[j][0:FPC, :])
        else:
            nc.gpsimd.collective_compute(
                "ReduceScatter", mybir.AluOpType.add,
                replica_groups=[list(range(N_CORES))],
                ins=[ypart[j][:]], outs=[yrs[j][:]])
        nc.sync.dma_start(yT[:, j * TQB:(j + 1) * TQB], yrs[j][:])

    if phases == 3 and rs_mode == "single":
        ypart_full, yrs_full = rs_full
        if sim_single_core:
            nc.sync.dma_start(yrs_full[:], ypart_full[0:FPC, :])
        else:
            nc.gpsimd.collective_compute(
                "ReduceScatter", mybir.AluOpType.add,
                replica_groups=[list(range(N_CORES))],
                ins=[ypart_full[:]], outs=[yrs_full[:]])
        nc.sync.dma_start(yT[:], yrs_full[:])


def make_mask_band() -> np.ndarray:
    p = np.arange(128)[:, None]
    j = np.arange(896)[None, :]
    return (p <= j - 384).astype(np.float32)


def prepare_in_maps(x, W_attn, W_proj):
    x = np.ascontiguousarray(np.asarray(x, dtype=np.float32))
    W_attn = np.ascontiguousarray(np.asarray(W_attn, dtype=np.float32))
    W_proj = np.ascontiguousarray(np.asarray(W_proj, dtype=np.float32))
    xT = np.ascontiguousarray(x.T.astype(NPBF))
    mask = make_mask_band().astype(NPBF)
    in_maps = []
    for r in range(N_CORES):
        rows = slice(r * FPC, (r + 1) * FPC)
        w_qkv = np.concatenate(
            [W_attn[0 * D:][rows], W_attn[1 * D:][rows], W_attn[2 * D:][rows]],
            axis=0)                                   # [768, D]
        in_maps.append({
            "xT": xT,
            "wqkvT": np.ascontiguousarray(w_qkv.T.astype(NPBF)),   # [D, 768]
            "wpT": np.ascontiguousarray(W_proj[:, rows].T.astype(NPBF)),  # [256, D]
            "maskB": mask,
        })
    return in_maps


def postprocess(results) -> np.ndarray:
    return np.concatenate(
        [np.asarray(r["yT"]).astype(np.float32).T for r in results], axis=1)


def kernel(x, W_attn, W_proj) -> np.ndarray:
    nc = build_nc()
    in_maps = prepare_in_maps(x, W_attn, W_proj)
    res = bass_utils.run_bass_kernel_spmd(
        nc, in_maps, core_ids=list(range(N_CORES)), trace=False)
    return postprocess(res.results)

